# revision 3
# baseline (speedup 1.0000x reference)
"""DGVAE GraphSAGE kernel for Trainium2 (8 NeuronCores, data-parallel over seeds).

Self-contained: hardcodes shapes/sharding. Host side replicates the reference's
RNG (jax key 42: neighbor-column permutations p1/p2 and eps), computes the
sampled neighbor trees (nb1/nb2) with numpy, and shards the 4096-seed batch as
512 seeds/core. Device side gathers normalized feature rows (gc = features /
(25*degrees)) with indirect DMAs, aggregates (tree-adds + selection matmuls),
and runs both GraphSAGE layers, the reparameterized sample, the decoder MLP and
softmax. Output [4096, 50] f32.
"""

import sys
import types

import numpy as np

# ---- optional NTFF profiling hook shim (for trace=True under axon) ----------
try:  # never let this break plain kernel() calls
    import antenv

    if "antenv.axon_hooks" not in sys.modules:
        _mod = types.ModuleType("antenv.axon_hooks")
        _hook_box = [None]
        _mod.set_axon_ntff_profile_hook = lambda h: _hook_box.__setitem__(0, h)
        _mod.get_axon_ntff_profile_hook = lambda: _hook_box[0]
        sys.modules["antenv.axon_hooks"] = _mod
        antenv.axon_hooks = _mod
        try:
            from trn_agent_boot.trn_boot import _ntff_profile_via_ctypes

            _mod.set_axon_ntff_profile_hook(
                _ntff_profile_via_ctypes("/opt/axon/libaxon_pjrt.so")
            )
        except Exception:
            pass
except Exception:
    pass

import concourse.bass as bass
import concourse.bacc as bacc
import concourse.mybir as mybir
import concourse.tile as tile
import concourse.bass_utils as bass_utils
from concourse.masks import make_identity

try:
    bass_utils.upload_artifacts = lambda tmpdir: f"local:{tmpdir}"
except Exception:
    pass

# ---- problem constants ------------------------------------------------------
N, B, F, MAXDEG = 100000, 4096, 128, 128
NEI1, NEI0 = 10, 25  # hop-1 samples/seed, hop-2 samples/hop-1 node
E1, E2, DEC, C = 128, 64, 256, 50
NCORES = 8
BC = B // NCORES          # 512 seeds per core
SG = BC // 128            # 4 seed groups of 128
M1 = BC * NEI1            # 5120 hop-1 nodes per core
NG = M1 // 128            # 40 hop-1 groups of 128
TPG = 1280 // 128         # 10 tiles per seed-group span of hop-1 nodes

F32 = mybir.dt.float32
I32 = mybir.dt.int32

_NC = None  # cached compiled Bass module


def _build_nc():
    nc = bacc.Bacc("TRN2", target_bir_lowering=False, debug=False, num_devices=NCORES)

    gc_d = nc.dram_tensor("gc", [N, F], F32, kind="ExternalInput")
    idx2_d = nc.dram_tensor("idx2", [128, NG * NEI0], I32, kind="ExternalInput")
    idx1_d = nc.dram_tensor("idx1", [128, NG], I32, kind="ExternalInput")
    idx0_d = nc.dram_tensor("idx0", [128, SG], I32, kind="ExternalInput")
    deg1s_d = nc.dram_tensor("deg1s", [128, NG], F32, kind="ExternalInput")
    deg0s_d = nc.dram_tensor("deg0s", [128, SG], F32, kind="ExternalInput")
    w1col_d = nc.dram_tensor("w1col", [128, NG], F32, kind="ExternalInput")
    sel_d = nc.dram_tensor("sel", [128, TPG * 128], F32, kind="ExternalInput")
    eps_d = nc.dram_tensor("eps", [128, SG * E2], F32, kind="ExternalInput")
    w0_d = nc.dram_tensor("w0", [F, E1], F32, kind="ExternalInput")
    wm_d = nc.dram_tensor("wm", [E1, E2], F32, kind="ExternalInput")
    ws_d = nc.dram_tensor("ws", [E1, E2], F32, kind="ExternalInput")
    w1_d = nc.dram_tensor("w1", [E2, DEC], F32, kind="ExternalInput")
    w2_d = nc.dram_tensor("w2", [128, 2 * C], F32, kind="ExternalInput")
    b1_d = nc.dram_tensor("b1", [128, 2], F32, kind="ExternalInput")
    b2_d = nc.dram_tensor("b2", [128, C], F32, kind="ExternalInput")
    out_d = nc.dram_tensor("out", [BC, C], F32, kind="ExternalOutput")

    with tile.TileContext(nc) as tc:
        with (
            tc.tile_pool(name="const", bufs=1) as cp,
            tc.tile_pool(name="g2", bufs=4) as g2p,
            tc.tile_pool(name="agg", bufs=3) as aggp,
            tc.tile_pool(name="wsel", bufs=3) as wselp,
            tc.tile_pool(name="small", bufs=3) as smp,
            tc.tile_pool(name="ppt", bufs=2, space="PSUM") as ppt,
            tc.tile_pool(name="ppm", bufs=2, space="PSUM") as ppm,
            tc.tile_pool(name="ppa", bufs=2, space="PSUM") as ppa,
        ):
            # ---- constant / input loads (HWDGE) ----
            idx2_sb = cp.tile([128, NG * NEI0], I32)
            nc.sync.dma_start(out=idx2_sb[:], in_=idx2_d[:])
            idx1_sb = cp.tile([128, NG], I32)
            nc.sync.dma_start(out=idx1_sb[:], in_=idx1_d[:])
            idx0_sb = cp.tile([128, SG], I32)
            nc.sync.dma_start(out=idx0_sb[:], in_=idx0_d[:])
            deg1s_sb = cp.tile([128, NG], F32)
            nc.sync.dma_start(out=deg1s_sb[:], in_=deg1s_d[:])
            deg0s_sb = cp.tile([128, SG], F32)
            nc.sync.dma_start(out=deg0s_sb[:], in_=deg0s_d[:])
            w1col_sb = cp.tile([128, NG], F32)
            nc.sync.dma_start(out=w1col_sb[:], in_=w1col_d[:])
            sel_sb = cp.tile([128, TPG * 128], F32)
            nc.sync.dma_start(out=sel_sb[:], in_=sel_d[:])
            eps_sb = cp.tile([128, SG * E2], F32)
            nc.sync.dma_start(out=eps_sb[:], in_=eps_d[:])
            w0_sb = cp.tile([F, E1], F32)
            nc.sync.dma_start(out=w0_sb[:], in_=w0_d[:])
            wm_sb = cp.tile([E1, E2], F32)
            nc.sync.dma_start(out=wm_sb[:], in_=wm_d[:])
            ws_sb = cp.tile([E1, E2], F32)
            nc.sync.dma_start(out=ws_sb[:], in_=ws_d[:])
            w1_sb = cp.tile([E2, DEC], F32)
            nc.sync.dma_start(out=w1_sb[:], in_=w1_d[:])
            w2_sb = cp.tile([128, 2 * C], F32)
            nc.sync.dma_start(out=w2_sb[:], in_=w2_d[:])
            b1_sb = cp.tile([128, 2], F32)
            nc.sync.dma_start(out=b1_sb[:], in_=b1_d[:])
            b2_sb = cp.tile([128, C], F32)
            nc.sync.dma_start(out=b2_sb[:], in_=b2_d[:])

            ident = cp.tile([128, 128], F32)
            make_identity(nc, ident[:])

            # persistent activations
            g1_all = cp.tile([128, NG * F], F32)   # gc rows of hop-1 nodes
            x1_all = cp.tile([128, NG * F], F32)   # raw features of hop-1 nodes
            g0_all = cp.tile([128, SG * F], F32)
            x0_all = cp.tile([128, SG * F], F32)
            h1_all = cp.tile([128, NG * E1], F32)
            h0_all = cp.tile([128, SG * E1], F32)

            def gather128(dst_ap, idx_col_ap):
                nc.gpsimd.indirect_dma_start(
                    out=dst_ap,
                    out_offset=None,
                    in_=gc_d[:],
                    in_offset=bass.IndirectOffsetOnAxis(ap=idx_col_ap, axis=0),
                )

            # hop-1 / hop-0 self rows
            for gi in range(NG):
                gather128(g1_all[:, gi * F : (gi + 1) * F], idx1_sb[:, gi : gi + 1])
                nc.scalar.mul(
                    x1_all[:, gi * F : (gi + 1) * F],
                    g1_all[:, gi * F : (gi + 1) * F],
                    deg1s_sb[:, gi : gi + 1],
                )
            for G in range(SG):
                gather128(g0_all[:, G * F : (G + 1) * F], idx0_sb[:, G : G + 1])
                nc.scalar.mul(
                    x0_all[:, G * F : (G + 1) * F],
                    g0_all[:, G * F : (G + 1) * F],
                    deg0s_sb[:, G : G + 1],
                )

            def hop2_group(gi):
                """h1 for hop-1 nodes [gi*128, (gi+1)*128)."""
                g2t = g2p.tile([128, NEI0 * F], F32, tag="g2t")
                for s in range(NEI0):
                    col = gi * NEI0 + s
                    gather128(g2t[:, s * F : (s + 1) * F], idx2_sb[:, col : col + 1])
                # tree-reduce 25 slices into slice 0
                n = NEI0
                while n > 1:
                    lo = n // 2
                    hi = n - lo  # slices [hi, n) added onto [0, lo)
                    nc.vector.tensor_add(
                        out=g2t[:, 0 : lo * F],
                        in0=g2t[:, 0 : lo * F],
                        in1=g2t[:, hi * F : n * F],
                    )
                    n = hi
                agg = aggp.tile([128, F], F32, tag="agg")
                nc.vector.tensor_add(
                    out=agg[:],
                    in0=g2t[:, 0:F],
                    in1=x1_all[:, gi * F : (gi + 1) * F],
                )
                aggT_ps = ppt.tile([128, 128], F32, tag="tp")
                nc.tensor.transpose(out=aggT_ps[:], in_=agg[:], identity=ident[:])
                aggT = aggp.tile([128, 128], F32, tag="aggT")
                nc.scalar.copy(aggT[:], aggT_ps[:])
                h1_ps = ppm.tile([128, E1], F32, tag="mm")
                nc.tensor.matmul(
                    h1_ps[:], lhsT=aggT[:], rhs=w0_sb[:], start=True, stop=True
                )
                nc.scalar.activation(
                    h1_all[:, gi * E1 : (gi + 1) * E1],
                    h1_ps[:],
                    mybir.ActivationFunctionType.Tanh,
                )

            def seed_group(G):
                """h0, layer-1 heads, decoder + softmax for seeds [G*128,(G+1)*128)."""
                # ---- h0: mean over 10 hop-1 gc rows (+2.5 scale), + x0, @W0, tanh
                ps0 = ppa.tile([128, F], F32, tag="acc")
                for t in range(TPG):
                    gi = G * TPG + t
                    nc.tensor.matmul(
                        ps0[:],
                        lhsT=sel_sb[:, t * 128 : (t + 1) * 128],
                        rhs=g1_all[:, gi * F : (gi + 1) * F],
                        start=(t == 0),
                        stop=(t == TPG - 1),
                    )
                tmp0 = smp.tile([128, F], F32, tag="tmp0")
                nc.vector.tensor_scalar(
                    out=tmp0[:],
                    in0=ps0[:],
                    scalar1=2.5,
                    scalar2=None,
                    op0=mybir.AluOpType.mult,
                )
                agg0 = smp.tile([128, F], F32, tag="agg0")
                nc.vector.tensor_add(
                    out=agg0[:], in0=tmp0[:], in1=x0_all[:, G * F : (G + 1) * F]
                )
                agg0T_ps = ppt.tile([128, 128], F32, tag="tp")
                nc.tensor.transpose(out=agg0T_ps[:], in_=agg0[:], identity=ident[:])
                agg0T = smp.tile([128, 128], F32, tag="agg0T")
                nc.scalar.copy(agg0T[:], agg0T_ps[:])
                h0_ps = ppm.tile([128, E1], F32, tag="mm")
                nc.tensor.matmul(
                    h0_ps[:], lhsT=agg0T[:], rhs=w0_sb[:], start=True, stop=True
                )
                nc.scalar.activation(
                    h0_all[:, G * E1 : (G + 1) * E1],
                    h0_ps[:],
                    mybir.ActivationFunctionType.Tanh,
                )
                # ---- layer 1: a1 = h0 + sum_t (sel*w1col).T @ h1
                ps1 = ppa.tile([128, E1], F32, tag="acc")
                for t in range(TPG):
                    gi = G * TPG + t
                    wsel = wselp.tile([128, 128], F32, tag="wsel")
                    nc.vector.tensor_scalar(
                        out=wsel[:],
                        in0=sel_sb[:, t * 128 : (t + 1) * 128],
                        scalar1=w1col_sb[:, gi : gi + 1],
                        scalar2=None,
                        op0=mybir.AluOpType.mult,
                    )
                    nc.tensor.matmul(
                        ps1[:],
                        lhsT=wsel[:],
                        rhs=h1_all[:, gi * E1 : (gi + 1) * E1],
                        start=(t == 0),
                        stop=(t == TPG - 1),
                    )
                a1 = smp.tile([128, E1], F32, tag="a1")
                nc.vector.tensor_add(
                    out=a1[:], in0=ps1[:], in1=h0_all[:, G * E1 : (G + 1) * E1]
                )
                a1T_ps = ppt.tile([128, 128], F32, tag="tp")
                nc.tensor.transpose(out=a1T_ps[:], in_=a1[:], identity=ident[:])
                a1T = smp.tile([128, 128], F32, tag="a1T")
                nc.scalar.copy(a1T[:], a1T_ps[:])
                zm_ps = ppm.tile([128, E2], F32, tag="mm")
                nc.tensor.matmul(zm_ps[:], lhsT=a1T[:], rhs=wm_sb[:], start=True, stop=True)
                zs_ps = ppm.tile([128, E2], F32, tag="mm")
                nc.tensor.matmul(zs_ps[:], lhsT=a1T[:], rhs=ws_sb[:], start=True, stop=True)
                ezs = smp.tile([128, E2], F32, tag="ezs")
                nc.scalar.activation(ezs[:], zs_ps[:], mybir.ActivationFunctionType.Exp)
                zt = smp.tile([128, E2], F32, tag="zt")
                nc.vector.tensor_mul(
                    out=zt[:], in0=ezs[:], in1=eps_sb[:, G * E2 : (G + 1) * E2]
                )
                z = smp.tile([128, E2], F32, tag="z")
                nc.vector.tensor_add(out=z[:], in0=zm_ps[:], in1=zt[:])
                # ---- decoder: relu(z@W1+b1)@W2 + b2, softmax
                zT_ps = ppt.tile([E2, 128], F32, tag="tp")
                nc.tensor.transpose(out=zT_ps[:], in_=z[:], identity=ident[:])
                zT = smp.tile([E2, 128], F32, tag="zTs")
                nc.scalar.copy(zT[:], zT_ps[:])
                lg_ps = ppa.tile([128, C], F32, tag="acc")
                for ch in range(2):
                    r_ps = ppm.tile([128, 128], F32, tag="mm")
                    nc.tensor.matmul(
                        r_ps[:],
                        lhsT=w1_sb[:, ch * 128 : (ch + 1) * 128],
                        rhs=zT[:],
                        start=True,
                        stop=True,
                    )
                    r_sb = smp.tile([128, 128], F32, tag="rsb")
                    nc.scalar.activation(
                        r_sb[:],
                        r_ps[:],
                        mybir.ActivationFunctionType.Relu,
                        bias=b1_sb[:, ch : ch + 1],
                    )
                    nc.tensor.matmul(
                        lg_ps[:],
                        lhsT=r_sb[:],
                        rhs=w2_sb[:, ch * C : (ch + 1) * C],
                        start=(ch == 0),
                        stop=(ch == 1),
                    )
                lg = smp.tile([128, C], F32, tag="lgsb")
                nc.vector.tensor_add(out=lg[:], in0=lg_ps[:], in1=b2_sb[:])
                nm = smp.tile([128, 1], F32, tag="nm")
                nc.vector.reduce_max(
                    out=nm[:], in_=lg[:], axis=mybir.AxisListType.X, negate=True
                )
                ex = smp.tile([128, C], F32, tag="ex")
                nc.scalar.activation(
                    ex[:], lg[:], mybir.ActivationFunctionType.Exp, bias=nm[:, 0:1]
                )
                ssum = smp.tile([128, 1], F32, tag="ssum")
                nc.vector.reduce_sum(out=ssum[:], in_=ex[:], axis=mybir.AxisListType.X)
                rinv = smp.tile([128, 1], F32, tag="rinv")
                nc.vector.reciprocal(rinv[:], ssum[:])
                o_sb = smp.tile([128, C], F32, tag="osb")
                nc.vector.tensor_scalar(
                    out=o_sb[:],
                    in0=ex[:],
                    scalar1=rinv[:, 0:1],
                    scalar2=None,
                    op0=mybir.AluOpType.mult,
                )
                nc.sync.dma_start(out=out_d[G * 128 : (G + 1) * 128, :], in_=o_sb[:])

            # interleave: 10 hop-2 groups, then their seed group
            for G in range(SG):
                for t in range(TPG):
                    hop2_group(G * TPG + t)
                seed_group(G)

    nc.compile()
    return nc


def _get_nc():
    global _NC
    if _NC is None:
        _NC = _build_nc()
    return _NC


def _host_prep(nodes, adj_info, degrees, features, W0, Wm, Ws, W1, b1, W2, b2):
    """Replicate reference RNG + build per-core input maps."""
    import jax

    nodes = np.asarray(nodes).astype(np.int64)
    adj_info = np.asarray(adj_info).astype(np.int64)
    degrees = np.asarray(degrees, dtype=np.float32)
    features = np.asarray(features, dtype=np.float32)
    W0 = np.ascontiguousarray(np.asarray(W0, dtype=np.float32))
    Wm = np.ascontiguousarray(np.asarray(Wm, dtype=np.float32))
    Ws = np.ascontiguousarray(np.asarray(Ws, dtype=np.float32))
    W1 = np.ascontiguousarray(np.asarray(W1, dtype=np.float32))
    b1 = np.asarray(b1, dtype=np.float32)
    W2 = np.asarray(W2, dtype=np.float32)
    b2 = np.asarray(b2, dtype=np.float32)

    with jax.default_device(jax.devices("cpu")[0]):
        key = jax.random.key(42)
        k1, k2, k3 = jax.random.split(key, 3)
        p1 = np.asarray(jax.random.permutation(k1, MAXDEG))[:NEI1].astype(np.int64)
        p2 = np.asarray(jax.random.permutation(k2, MAXDEG))[:NEI0].astype(np.int64)
        eps = np.asarray(
            jax.random.normal(k3, (B, E2), dtype=np.float32), dtype=np.float32
        )

    gc = (features / (NEI0 * degrees[:, None])).astype(np.float32)
    gc = np.ascontiguousarray(gc)

    # selection matrix: sel[i, t*128+s] = 2.5 if (t*128+i)//10 == s  -> NO:
    # scale 2.5 applied on-device; entries are 0/1
    sel = np.zeros((128, TPG * 128), dtype=np.float32)
    for t in range(TPG):
        i = np.arange(128)
        s = (t * 128 + i) // NEI1
        sel[i, t * 128 + s] = 1.0

    w2r = np.zeros((128, 2 * C), dtype=np.float32)
    w2r[:, :C] = W2[:128]
    w2r[:, C:] = W2[128:]
    b1r = np.stack([b1[:128], b1[128:]], axis=1).astype(np.float32)
    b1r = np.ascontiguousarray(b1r)
    b2r = np.broadcast_to(b2[None, :], (128, C)).astype(np.float32)
    b2r = np.ascontiguousarray(b2r)

    in_maps = []
    for c in range(NCORES):
        nodes_c = nodes[c * BC : (c + 1) * BC]
        nb1 = adj_info[nodes_c][:, p1].reshape(-1)  # [5120]
        nb2 = adj_info[nb1[:, None], p2[None, :]]  # [5120, 25]

        idx1 = nb1.reshape(NG, 128).T.astype(np.int32)  # [128, NG]
        idx0 = nodes_c.reshape(SG, 128).T.astype(np.int32)  # [128, SG]
        # idx2[p, gi*25+s] = nb2[gi*128+p, s]
        idx2 = (
            nb2.reshape(NG, 128, NEI0).transpose(1, 0, 2).reshape(128, NG * NEI0)
        ).astype(np.int32)

        deg_nb1 = degrees[nb1]  # [5120]
        deg1s = (NEI0 * deg_nb1).reshape(NG, 128).T.astype(np.float32)
        w1col = (0.1 / deg_nb1).reshape(NG, 128).T.astype(np.float32)
        deg0s = (NEI0 * degrees[nodes_c]).reshape(SG, 128).T.astype(np.float32)

        eps_c = (
            eps[c * BC : (c + 1) * BC].reshape(SG, 128, E2).transpose(1, 0, 2)
        ).reshape(128, SG * E2)

        in_maps.append(
            {
                "gc": gc,
                "idx2": np.ascontiguousarray(idx2),
                "idx1": np.ascontiguousarray(idx1),
                "idx0": np.ascontiguousarray(idx0),
                "deg1s": np.ascontiguousarray(deg1s),
                "deg0s": np.ascontiguousarray(deg0s),
                "w1col": np.ascontiguousarray(w1col),
                "sel": sel,
                "eps": np.ascontiguousarray(eps_c.astype(np.float32)),
                "w0": W0,
                "wm": Wm,
                "ws": Ws,
                "w1": W1,
                "w2": w2r,
                "b1": b1r,
                "b2": b2r,
            }
        )
    return in_maps


def _run(in_maps, trace=False):
    nc = _get_nc()
    res = bass_utils.run_bass_kernel_spmd(
        nc, in_maps, core_ids=list(range(NCORES)), trace=trace
    )
    out = np.concatenate([r["out"] for r in res.results], axis=0)
    return out.astype(np.float32), res


def kernel(**inputs) -> np.ndarray:
    in_maps = _host_prep(**inputs)
    out, _ = _run(in_maps, trace=False)
    return out


def kernel_traced(**inputs):
    """Returns (output, exec_time_ns) using NTFF profiling."""
    in_maps = _host_prep(**inputs)
    out, res = _run(in_maps, trace=True)
    return out, res.exec_time_ns


# revision 5
# speedup vs baseline: 1.0215x; 1.0215x over previous
"""DGVAE GraphSAGE kernel for Trainium2 (8 NeuronCores, data-parallel over seeds).

Self-contained: hardcodes shapes/sharding. Host side replicates the reference's
RNG (jax key 42: neighbor-column permutations p1/p2 and eps), computes the
sampled neighbor trees (nb1/nb2) with numpy, and shards the 4096-seed batch as
512 seeds/core. Device side gathers normalized feature rows (gc = features /
(25*degrees)) with indirect DMAs, aggregates (tree-adds + selection matmuls),
and runs both GraphSAGE layers, the reparameterized sample, the decoder MLP and
softmax. Output [4096, 50] f32.
"""

import sys
import types

import numpy as np

# ---- optional NTFF profiling hook shim (for trace=True under axon) ----------
try:  # never let this break plain kernel() calls
    import antenv

    if "antenv.axon_hooks" not in sys.modules:
        _mod = types.ModuleType("antenv.axon_hooks")
        _hook_box = [None]
        _mod.set_axon_ntff_profile_hook = lambda h: _hook_box.__setitem__(0, h)
        _mod.get_axon_ntff_profile_hook = lambda: _hook_box[0]
        sys.modules["antenv.axon_hooks"] = _mod
        antenv.axon_hooks = _mod
        try:
            from trn_agent_boot.trn_boot import _ntff_profile_via_ctypes

            _mod.set_axon_ntff_profile_hook(
                _ntff_profile_via_ctypes("/opt/axon/libaxon_pjrt.so")
            )
        except Exception:
            pass
except Exception:
    pass

import concourse.bass as bass
import concourse.bacc as bacc
import concourse.mybir as mybir
import concourse.tile as tile
import concourse.bass_utils as bass_utils
from concourse.masks import make_identity

try:
    bass_utils.upload_artifacts = lambda tmpdir: f"local:{tmpdir}"
except Exception:
    pass

# ---- problem constants ------------------------------------------------------
N, B, F, MAXDEG = 100000, 4096, 128, 128
NEI1, NEI0 = 10, 25  # hop-1 samples/seed, hop-2 samples/hop-1 node
E1, E2, DEC, C = 128, 64, 256, 50
NCORES = 8
BC = B // NCORES          # 512 seeds per core
SG = BC // 128            # 4 seed groups of 128
M1 = BC * NEI1            # 5120 hop-1 nodes per core
NG = M1 // 128            # 40 hop-1 groups of 128
TPG = 1280 // 128         # 10 tiles per seed-group span of hop-1 nodes

F32 = mybir.dt.float32
I32 = mybir.dt.int32

_NC = None  # cached compiled Bass module


def _build_nc():
    nc = bacc.Bacc("TRN2", target_bir_lowering=False, debug=False, num_devices=NCORES)

    gc_d = nc.dram_tensor("gc", [N, F], F32, kind="ExternalInput")
    idx2_d = nc.dram_tensor("idx2", [128, NG * NEI0], I32, kind="ExternalInput")
    idx1_d = nc.dram_tensor("idx1", [128, NG], I32, kind="ExternalInput")
    idx0_d = nc.dram_tensor("idx0", [128, SG], I32, kind="ExternalInput")
    deg1s_d = nc.dram_tensor("deg1s", [128, NG], F32, kind="ExternalInput")
    deg0s_d = nc.dram_tensor("deg0s", [128, SG], F32, kind="ExternalInput")
    w1col_d = nc.dram_tensor("w1col", [128, NG], F32, kind="ExternalInput")
    sel_d = nc.dram_tensor("sel", [128, TPG * 128], F32, kind="ExternalInput")
    eps_d = nc.dram_tensor("eps", [128, SG * E2], F32, kind="ExternalInput")
    w0_d = nc.dram_tensor("w0", [F, E1], F32, kind="ExternalInput")
    wm_d = nc.dram_tensor("wm", [E1, E2], F32, kind="ExternalInput")
    ws_d = nc.dram_tensor("ws", [E1, E2], F32, kind="ExternalInput")
    w1_d = nc.dram_tensor("w1", [E2, DEC], F32, kind="ExternalInput")
    w2_d = nc.dram_tensor("w2", [128, 2 * C], F32, kind="ExternalInput")
    b1_d = nc.dram_tensor("b1", [128, 2], F32, kind="ExternalInput")
    b2_d = nc.dram_tensor("b2", [128, C], F32, kind="ExternalInput")
    out_d = nc.dram_tensor("out", [BC, C], F32, kind="ExternalOutput")

    with tile.TileContext(nc) as tc:
        with (
            tc.tile_pool(name="const", bufs=1) as cp,
            tc.tile_pool(name="g2", bufs=6) as g2p,
            tc.tile_pool(name="agg", bufs=3) as aggp,
            tc.tile_pool(name="wsel", bufs=3) as wselp,
            tc.tile_pool(name="small", bufs=3) as smp,
            tc.tile_pool(name="ppt", bufs=2, space="PSUM") as ppt,
            tc.tile_pool(name="ppm", bufs=2, space="PSUM") as ppm,
            tc.tile_pool(name="ppa", bufs=2, space="PSUM") as ppa,
        ):
            # ---- constant / input loads (HWDGE) ----
            idx2_sb = cp.tile([128, NG * NEI0], I32)
            nc.sync.dma_start(out=idx2_sb[:], in_=idx2_d[:])
            idx1_sb = cp.tile([128, NG], I32)
            nc.sync.dma_start(out=idx1_sb[:], in_=idx1_d[:])
            idx0_sb = cp.tile([128, SG], I32)
            nc.sync.dma_start(out=idx0_sb[:], in_=idx0_d[:])
            deg1s_sb = cp.tile([128, NG], F32)
            nc.sync.dma_start(out=deg1s_sb[:], in_=deg1s_d[:])
            deg0s_sb = cp.tile([128, SG], F32)
            nc.sync.dma_start(out=deg0s_sb[:], in_=deg0s_d[:])
            w1col_sb = cp.tile([128, NG], F32)
            nc.sync.dma_start(out=w1col_sb[:], in_=w1col_d[:])
            sel_sb = cp.tile([128, TPG * 128], F32)
            nc.sync.dma_start(out=sel_sb[:], in_=sel_d[:])
            eps_sb = cp.tile([128, SG * E2], F32)
            nc.sync.dma_start(out=eps_sb[:], in_=eps_d[:])
            w0_sb = cp.tile([F, E1], F32)
            nc.sync.dma_start(out=w0_sb[:], in_=w0_d[:])
            wm_sb = cp.tile([E1, E2], F32)
            nc.sync.dma_start(out=wm_sb[:], in_=wm_d[:])
            ws_sb = cp.tile([E1, E2], F32)
            nc.sync.dma_start(out=ws_sb[:], in_=ws_d[:])
            w1_sb = cp.tile([E2, DEC], F32)
            nc.sync.dma_start(out=w1_sb[:], in_=w1_d[:])
            w2_sb = cp.tile([128, 2 * C], F32)
            nc.sync.dma_start(out=w2_sb[:], in_=w2_d[:])
            b1_sb = cp.tile([128, 2], F32)
            nc.sync.dma_start(out=b1_sb[:], in_=b1_d[:])
            b2_sb = cp.tile([128, C], F32)
            nc.sync.dma_start(out=b2_sb[:], in_=b2_d[:])

            ident = cp.tile([128, 128], F32)
            make_identity(nc, ident[:])

            # persistent activations
            g1_all = cp.tile([128, NG * F], F32)   # gc rows of hop-1 nodes
            x1_all = cp.tile([128, NG * F], F32)   # raw features of hop-1 nodes
            g0_all = cp.tile([128, SG * F], F32)
            x0_all = cp.tile([128, SG * F], F32)
            h1_all = cp.tile([128, NG * E1], F32)
            h0_all = cp.tile([128, SG * E1], F32)

            def gather128(dst_ap, idx_col_ap):
                nc.gpsimd.indirect_dma_start(
                    out=dst_ap,
                    out_offset=None,
                    in_=gc_d[:],
                    in_offset=bass.IndirectOffsetOnAxis(ap=idx_col_ap, axis=0),
                )

            # hop-1 / hop-0 self rows
            for gi in range(NG):
                gather128(g1_all[:, gi * F : (gi + 1) * F], idx1_sb[:, gi : gi + 1])
                nc.scalar.mul(
                    x1_all[:, gi * F : (gi + 1) * F],
                    g1_all[:, gi * F : (gi + 1) * F],
                    deg1s_sb[:, gi : gi + 1],
                )
            for G in range(SG):
                gather128(g0_all[:, G * F : (G + 1) * F], idx0_sb[:, G : G + 1])
                nc.scalar.mul(
                    x0_all[:, G * F : (G + 1) * F],
                    g0_all[:, G * F : (G + 1) * F],
                    deg0s_sb[:, G : G + 1],
                )

            def hop2_group(gi):
                """h1 for hop-1 nodes [gi*128, (gi+1)*128)."""
                g2t = g2p.tile([128, NEI0 * F], F32, tag="g2t")
                for s in range(NEI0):
                    col = gi * NEI0 + s
                    gather128(g2t[:, s * F : (s + 1) * F], idx2_sb[:, col : col + 1])
                # one strided reduce over the sample axis: [p, (s f)] -> [p, f, s]
                agg = aggp.tile([128, F], F32, tag="agg")
                nc.vector.reduce_sum(
                    out=agg[:],
                    in_=g2t[:].rearrange("p (s f) -> p f s", f=F),
                    axis=mybir.AxisListType.X,
                )
                nc.vector.tensor_add(
                    out=agg[:],
                    in0=agg[:],
                    in1=x1_all[:, gi * F : (gi + 1) * F],
                )
                aggT_ps = ppt.tile([128, 128], F32, tag="tp")
                nc.tensor.transpose(out=aggT_ps[:], in_=agg[:], identity=ident[:])
                aggT = aggp.tile([128, 128], F32, tag="aggT")
                nc.scalar.copy(aggT[:], aggT_ps[:])
                h1_ps = ppm.tile([128, E1], F32, tag="mm")
                nc.tensor.matmul(
                    h1_ps[:], lhsT=aggT[:], rhs=w0_sb[:], start=True, stop=True
                )
                nc.scalar.activation(
                    h1_all[:, gi * E1 : (gi + 1) * E1],
                    h1_ps[:],
                    mybir.ActivationFunctionType.Tanh,
                )

            def seed_group(G):
                """h0, layer-1 heads, decoder + softmax for seeds [G*128,(G+1)*128)."""
                # ---- h0: mean over 10 hop-1 gc rows (+2.5 scale), + x0, @W0, tanh
                ps0 = ppa.tile([128, F], F32, tag="acc")
                for t in range(TPG):
                    gi = G * TPG + t
                    nc.tensor.matmul(
                        ps0[:],
                        lhsT=sel_sb[:, t * 128 : (t + 1) * 128],
                        rhs=g1_all[:, gi * F : (gi + 1) * F],
                        start=(t == 0),
                        stop=(t == TPG - 1),
                    )
                tmp0 = smp.tile([128, F], F32, tag="tmp0")
                nc.vector.tensor_scalar(
                    out=tmp0[:],
                    in0=ps0[:],
                    scalar1=2.5,
                    scalar2=None,
                    op0=mybir.AluOpType.mult,
                )
                agg0 = smp.tile([128, F], F32, tag="agg0")
                nc.vector.tensor_add(
                    out=agg0[:], in0=tmp0[:], in1=x0_all[:, G * F : (G + 1) * F]
                )
                agg0T_ps = ppt.tile([128, 128], F32, tag="tp")
                nc.tensor.transpose(out=agg0T_ps[:], in_=agg0[:], identity=ident[:])
                agg0T = smp.tile([128, 128], F32, tag="agg0T")
                nc.scalar.copy(agg0T[:], agg0T_ps[:])
                h0_ps = ppm.tile([128, E1], F32, tag="mm")
                nc.tensor.matmul(
                    h0_ps[:], lhsT=agg0T[:], rhs=w0_sb[:], start=True, stop=True
                )
                nc.scalar.activation(
                    h0_all[:, G * E1 : (G + 1) * E1],
                    h0_ps[:],
                    mybir.ActivationFunctionType.Tanh,
                )
                # ---- layer 1: a1 = h0 + sum_t (sel*w1col).T @ h1
                ps1 = ppa.tile([128, E1], F32, tag="acc")
                for t in range(TPG):
                    gi = G * TPG + t
                    wsel = wselp.tile([128, 128], F32, tag="wsel")
                    nc.scalar.mul(
                        wsel[:],
                        sel_sb[:, t * 128 : (t + 1) * 128],
                        w1col_sb[:, gi : gi + 1],
                    )
                    nc.tensor.matmul(
                        ps1[:],
                        lhsT=wsel[:],
                        rhs=h1_all[:, gi * E1 : (gi + 1) * E1],
                        start=(t == 0),
                        stop=(t == TPG - 1),
                    )
                a1 = smp.tile([128, E1], F32, tag="a1")
                nc.vector.tensor_add(
                    out=a1[:], in0=ps1[:], in1=h0_all[:, G * E1 : (G + 1) * E1]
                )
                a1T_ps = ppt.tile([128, 128], F32, tag="tp")
                nc.tensor.transpose(out=a1T_ps[:], in_=a1[:], identity=ident[:])
                a1T = smp.tile([128, 128], F32, tag="a1T")
                nc.scalar.copy(a1T[:], a1T_ps[:])
                zm_ps = ppm.tile([128, E2], F32, tag="mm")
                nc.tensor.matmul(zm_ps[:], lhsT=a1T[:], rhs=wm_sb[:], start=True, stop=True)
                zs_ps = ppm.tile([128, E2], F32, tag="mm")
                nc.tensor.matmul(zs_ps[:], lhsT=a1T[:], rhs=ws_sb[:], start=True, stop=True)
                ezs = smp.tile([128, E2], F32, tag="ezs")
                nc.scalar.activation(ezs[:], zs_ps[:], mybir.ActivationFunctionType.Exp)
                zt = smp.tile([128, E2], F32, tag="zt")
                nc.vector.tensor_mul(
                    out=zt[:], in0=ezs[:], in1=eps_sb[:, G * E2 : (G + 1) * E2]
                )
                z = smp.tile([128, E2], F32, tag="z")
                nc.vector.tensor_add(out=z[:], in0=zm_ps[:], in1=zt[:])
                # ---- decoder: relu(z@W1+b1)@W2 + b2, softmax
                zT_ps = ppt.tile([E2, 128], F32, tag="tp")
                nc.tensor.transpose(out=zT_ps[:], in_=z[:], identity=ident[:])
                zT = smp.tile([E2, 128], F32, tag="zTs")
                nc.scalar.copy(zT[:], zT_ps[:])
                lg_ps = ppa.tile([128, C], F32, tag="acc")
                for ch in range(2):
                    r_ps = ppm.tile([128, 128], F32, tag="mm")
                    nc.tensor.matmul(
                        r_ps[:],
                        lhsT=w1_sb[:, ch * 128 : (ch + 1) * 128],
                        rhs=zT[:],
                        start=True,
                        stop=True,
                    )
                    r_sb = smp.tile([128, 128], F32, tag="rsb")
                    nc.scalar.activation(
                        r_sb[:],
                        r_ps[:],
                        mybir.ActivationFunctionType.Relu,
                        bias=b1_sb[:, ch : ch + 1],
                    )
                    nc.tensor.matmul(
                        lg_ps[:],
                        lhsT=r_sb[:],
                        rhs=w2_sb[:, ch * C : (ch + 1) * C],
                        start=(ch == 0),
                        stop=(ch == 1),
                    )
                lg = smp.tile([128, C], F32, tag="lgsb")
                nc.vector.tensor_add(out=lg[:], in0=lg_ps[:], in1=b2_sb[:])
                nm = smp.tile([128, 1], F32, tag="nm")
                nc.vector.reduce_max(
                    out=nm[:], in_=lg[:], axis=mybir.AxisListType.X, negate=True
                )
                ex = smp.tile([128, C], F32, tag="ex")
                nc.scalar.activation(
                    ex[:], lg[:], mybir.ActivationFunctionType.Exp, bias=nm[:, 0:1]
                )
                ssum = smp.tile([128, 1], F32, tag="ssum")
                nc.vector.reduce_sum(out=ssum[:], in_=ex[:], axis=mybir.AxisListType.X)
                rinv = smp.tile([128, 1], F32, tag="rinv")
                nc.vector.reciprocal(rinv[:], ssum[:])
                o_sb = smp.tile([128, C], F32, tag="osb")
                nc.vector.tensor_scalar(
                    out=o_sb[:],
                    in0=ex[:],
                    scalar1=rinv[:, 0:1],
                    scalar2=None,
                    op0=mybir.AluOpType.mult,
                )
                nc.sync.dma_start(out=out_d[G * 128 : (G + 1) * 128, :], in_=o_sb[:])

            # interleave: 10 hop-2 groups, then their seed group
            for G in range(SG):
                for t in range(TPG):
                    hop2_group(G * TPG + t)
                seed_group(G)

    nc.compile()
    return nc


def _get_nc():
    global _NC
    if _NC is None:
        _NC = _build_nc()
    return _NC


def _host_prep(nodes, adj_info, degrees, features, W0, Wm, Ws, W1, b1, W2, b2):
    """Replicate reference RNG + build per-core input maps."""
    import jax

    nodes = np.asarray(nodes).astype(np.int64)
    adj_info = np.asarray(adj_info).astype(np.int64)
    degrees = np.asarray(degrees, dtype=np.float32)
    features = np.asarray(features, dtype=np.float32)
    W0 = np.ascontiguousarray(np.asarray(W0, dtype=np.float32))
    Wm = np.ascontiguousarray(np.asarray(Wm, dtype=np.float32))
    Ws = np.ascontiguousarray(np.asarray(Ws, dtype=np.float32))
    W1 = np.ascontiguousarray(np.asarray(W1, dtype=np.float32))
    b1 = np.asarray(b1, dtype=np.float32)
    W2 = np.asarray(W2, dtype=np.float32)
    b2 = np.asarray(b2, dtype=np.float32)

    with jax.default_device(jax.devices("cpu")[0]):
        key = jax.random.key(42)
        k1, k2, k3 = jax.random.split(key, 3)
        p1 = np.asarray(jax.random.permutation(k1, MAXDEG))[:NEI1].astype(np.int64)
        p2 = np.asarray(jax.random.permutation(k2, MAXDEG))[:NEI0].astype(np.int64)
        eps = np.asarray(
            jax.random.normal(k3, (B, E2), dtype=np.float32), dtype=np.float32
        )

    gc = (features / (NEI0 * degrees[:, None])).astype(np.float32)
    gc = np.ascontiguousarray(gc)

    # selection matrix: sel[i, t*128+s] = 2.5 if (t*128+i)//10 == s  -> NO:
    # scale 2.5 applied on-device; entries are 0/1
    sel = np.zeros((128, TPG * 128), dtype=np.float32)
    for t in range(TPG):
        i = np.arange(128)
        s = (t * 128 + i) // NEI1
        sel[i, t * 128 + s] = 1.0

    w2r = np.zeros((128, 2 * C), dtype=np.float32)
    w2r[:, :C] = W2[:128]
    w2r[:, C:] = W2[128:]
    b1r = np.stack([b1[:128], b1[128:]], axis=1).astype(np.float32)
    b1r = np.ascontiguousarray(b1r)
    b2r = np.broadcast_to(b2[None, :], (128, C)).astype(np.float32)
    b2r = np.ascontiguousarray(b2r)

    in_maps = []
    for c in range(NCORES):
        nodes_c = nodes[c * BC : (c + 1) * BC]
        nb1 = adj_info[nodes_c][:, p1].reshape(-1)  # [5120]
        nb2 = adj_info[nb1[:, None], p2[None, :]]  # [5120, 25]

        idx1 = nb1.reshape(NG, 128).T.astype(np.int32)  # [128, NG]
        idx0 = nodes_c.reshape(SG, 128).T.astype(np.int32)  # [128, SG]
        # idx2[p, gi*25+s] = nb2[gi*128+p, s]
        idx2 = (
            nb2.reshape(NG, 128, NEI0).transpose(1, 0, 2).reshape(128, NG * NEI0)
        ).astype(np.int32)

        deg_nb1 = degrees[nb1]  # [5120]
        deg1s = (NEI0 * deg_nb1).reshape(NG, 128).T.astype(np.float32)
        w1col = (0.1 / deg_nb1).reshape(NG, 128).T.astype(np.float32)
        deg0s = (NEI0 * degrees[nodes_c]).reshape(SG, 128).T.astype(np.float32)

        eps_c = (
            eps[c * BC : (c + 1) * BC].reshape(SG, 128, E2).transpose(1, 0, 2)
        ).reshape(128, SG * E2)

        in_maps.append(
            {
                "gc": gc,
                "idx2": np.ascontiguousarray(idx2),
                "idx1": np.ascontiguousarray(idx1),
                "idx0": np.ascontiguousarray(idx0),
                "deg1s": np.ascontiguousarray(deg1s),
                "deg0s": np.ascontiguousarray(deg0s),
                "w1col": np.ascontiguousarray(w1col),
                "sel": sel,
                "eps": np.ascontiguousarray(eps_c.astype(np.float32)),
                "w0": W0,
                "wm": Wm,
                "ws": Ws,
                "w1": W1,
                "w2": w2r,
                "b1": b1r,
                "b2": b2r,
            }
        )
    return in_maps


def _run(in_maps, trace=False):
    nc = _get_nc()
    res = bass_utils.run_bass_kernel_spmd(
        nc, in_maps, core_ids=list(range(NCORES)), trace=trace
    )
    out = np.concatenate([r["out"] for r in res.results], axis=0)
    return out.astype(np.float32), res


def kernel(**inputs) -> np.ndarray:
    in_maps = _host_prep(**inputs)
    out, _ = _run(in_maps, trace=False)
    return out


def kernel_traced(**inputs):
    """Returns (output, exec_time_ns) using NTFF profiling."""
    in_maps = _host_prep(**inputs)
    out, res = _run(in_maps, trace=True)
    return out, res.exec_time_ns


# revision 7
# speedup vs baseline: 1.0250x; 1.0034x over previous
"""DGVAE GraphSAGE kernel for Trainium2 (8 NeuronCores, data-parallel over seeds).

Self-contained: hardcodes shapes/sharding. Host side replicates the reference's
RNG (jax key 42: neighbor-column permutations p1/p2 and eps), computes the
sampled neighbor trees (nb1/nb2) with numpy, and shards the 4096-seed batch as
512 seeds/core. Device side gathers normalized feature rows (gc = features /
(25*degrees)) with indirect DMAs, aggregates (tree-adds + selection matmuls),
and runs both GraphSAGE layers, the reparameterized sample, the decoder MLP and
softmax. Output [4096, 50] f32.
"""

import sys
import types

import numpy as np

# ---- optional NTFF profiling hook shim (for trace=True under axon) ----------
try:  # never let this break plain kernel() calls
    import antenv

    if "antenv.axon_hooks" not in sys.modules:
        _mod = types.ModuleType("antenv.axon_hooks")
        _hook_box = [None]
        _mod.set_axon_ntff_profile_hook = lambda h: _hook_box.__setitem__(0, h)
        _mod.get_axon_ntff_profile_hook = lambda: _hook_box[0]
        sys.modules["antenv.axon_hooks"] = _mod
        antenv.axon_hooks = _mod
        try:
            from trn_agent_boot.trn_boot import _ntff_profile_via_ctypes

            _mod.set_axon_ntff_profile_hook(
                _ntff_profile_via_ctypes("/opt/axon/libaxon_pjrt.so")
            )
        except Exception:
            pass
except Exception:
    pass

import concourse.bass as bass
import concourse.bacc as bacc
import concourse.mybir as mybir
import concourse.tile as tile
import concourse.bass_utils as bass_utils
from concourse.masks import make_identity

try:
    bass_utils.upload_artifacts = lambda tmpdir: f"local:{tmpdir}"
except Exception:
    pass

# ---- problem constants ------------------------------------------------------
N, B, F, MAXDEG = 100000, 4096, 128, 128
NEI1, NEI0 = 10, 25  # hop-1 samples/seed, hop-2 samples/hop-1 node
E1, E2, DEC, C = 128, 64, 256, 50
NCORES = 8
BC = B // NCORES          # 512 seeds per core
SG = BC // 128            # 4 seed groups of 128
M1 = BC * NEI1            # 5120 hop-1 nodes per core
NG = M1 // 128            # 40 hop-1 groups of 128
TPG = 1280 // 128         # 10 tiles per seed-group span of hop-1 nodes

F32 = mybir.dt.float32
I32 = mybir.dt.int32

_NC = None  # cached compiled Bass module


def _build_nc():
    nc = bacc.Bacc("TRN2", target_bir_lowering=False, debug=False, num_devices=NCORES)

    gc_d = nc.dram_tensor("gc", [N, F], F32, kind="ExternalInput")
    idx2_d = nc.dram_tensor("idx2", [128, NG * NEI0], I32, kind="ExternalInput")
    idx1_d = nc.dram_tensor("idx1", [128, NG], I32, kind="ExternalInput")
    idx0_d = nc.dram_tensor("idx0", [128, SG], I32, kind="ExternalInput")
    deg1s_d = nc.dram_tensor("deg1s", [128, NG], F32, kind="ExternalInput")
    deg0s_d = nc.dram_tensor("deg0s", [128, SG], F32, kind="ExternalInput")
    w1col_d = nc.dram_tensor("w1col", [128, NG], F32, kind="ExternalInput")
    sel_d = nc.dram_tensor("sel", [128, TPG * 128], F32, kind="ExternalInput")
    eps_d = nc.dram_tensor("eps", [128, SG * E2], F32, kind="ExternalInput")
    w0_d = nc.dram_tensor("w0", [F, E1], F32, kind="ExternalInput")
    wm_d = nc.dram_tensor("wm", [E1, E2], F32, kind="ExternalInput")
    ws_d = nc.dram_tensor("ws", [E1, E2], F32, kind="ExternalInput")
    w1_d = nc.dram_tensor("w1", [E2, DEC], F32, kind="ExternalInput")
    w2_d = nc.dram_tensor("w2", [128, 2 * C], F32, kind="ExternalInput")
    b1_d = nc.dram_tensor("b1", [128, 2], F32, kind="ExternalInput")
    b2_d = nc.dram_tensor("b2", [128, C], F32, kind="ExternalInput")
    out_d = nc.dram_tensor("out", [BC, C], F32, kind="ExternalOutput")

    with tile.TileContext(nc) as tc:
        with (
            tc.tile_pool(name="const", bufs=1) as cp,
            tc.tile_pool(name="g2", bufs=6) as g2p,
            tc.tile_pool(name="agg", bufs=3) as aggp,
            tc.tile_pool(name="wsel", bufs=3) as wselp,
            tc.tile_pool(name="small", bufs=3) as smp,
            tc.tile_pool(name="ppt", bufs=2, space="PSUM") as ppt,
            tc.tile_pool(name="ppm", bufs=2, space="PSUM") as ppm,
            tc.tile_pool(name="ppa", bufs=3, space="PSUM") as ppa,
        ):
            # ---- constant / input loads (HWDGE) ----
            idx2_sb = cp.tile([128, NG * NEI0], I32)
            nc.sync.dma_start(out=idx2_sb[:], in_=idx2_d[:])
            idx1_sb = cp.tile([128, NG], I32)
            nc.sync.dma_start(out=idx1_sb[:], in_=idx1_d[:])
            idx0_sb = cp.tile([128, SG], I32)
            nc.sync.dma_start(out=idx0_sb[:], in_=idx0_d[:])
            deg1s_sb = cp.tile([128, NG], F32)
            nc.sync.dma_start(out=deg1s_sb[:], in_=deg1s_d[:])
            deg0s_sb = cp.tile([128, SG], F32)
            nc.sync.dma_start(out=deg0s_sb[:], in_=deg0s_d[:])
            w1col_sb = cp.tile([128, NG], F32)
            nc.sync.dma_start(out=w1col_sb[:], in_=w1col_d[:])
            sel_sb = cp.tile([128, TPG * 128], F32)
            nc.sync.dma_start(out=sel_sb[:], in_=sel_d[:])
            eps_sb = cp.tile([128, SG * E2], F32)
            nc.sync.dma_start(out=eps_sb[:], in_=eps_d[:])
            w0_sb = cp.tile([F, E1], F32)
            nc.sync.dma_start(out=w0_sb[:], in_=w0_d[:])
            wm_sb = cp.tile([E1, E2], F32)
            nc.sync.dma_start(out=wm_sb[:], in_=wm_d[:])
            ws_sb = cp.tile([E1, E2], F32)
            nc.sync.dma_start(out=ws_sb[:], in_=ws_d[:])
            w1_sb = cp.tile([E2, DEC], F32)
            nc.sync.dma_start(out=w1_sb[:], in_=w1_d[:])
            w2_sb = cp.tile([128, 2 * C], F32)
            nc.sync.dma_start(out=w2_sb[:], in_=w2_d[:])
            b1_sb = cp.tile([128, 2], F32)
            nc.sync.dma_start(out=b1_sb[:], in_=b1_d[:])
            b2_sb = cp.tile([128, C], F32)
            nc.sync.dma_start(out=b2_sb[:], in_=b2_d[:])

            ident = cp.tile([128, 128], F32)
            make_identity(nc, ident[:])

            # persistent activations
            g1_all = cp.tile([128, NG * F], F32)   # gc rows of hop-1 nodes
            x1_all = cp.tile([128, NG * F], F32)   # raw features of hop-1 nodes
            g0_all = cp.tile([128, SG * F], F32)
            x0_all = cp.tile([128, SG * F], F32)
            h1_all = cp.tile([128, NG * E1], F32)
            h0_all = cp.tile([128, SG * E1], F32)

            def gather128(dst_ap, idx_col_ap):
                nc.gpsimd.indirect_dma_start(
                    out=dst_ap,
                    out_offset=None,
                    in_=gc_d[:],
                    in_offset=bass.IndirectOffsetOnAxis(ap=idx_col_ap, axis=0),
                )

            # hop-1 / hop-0 self rows
            for gi in range(NG):
                gather128(g1_all[:, gi * F : (gi + 1) * F], idx1_sb[:, gi : gi + 1])
                nc.scalar.mul(
                    x1_all[:, gi * F : (gi + 1) * F],
                    g1_all[:, gi * F : (gi + 1) * F],
                    deg1s_sb[:, gi : gi + 1],
                )
            for G in range(SG):
                gather128(g0_all[:, G * F : (G + 1) * F], idx0_sb[:, G : G + 1])
                nc.scalar.mul(
                    x0_all[:, G * F : (G + 1) * F],
                    g0_all[:, G * F : (G + 1) * F],
                    deg0s_sb[:, G : G + 1],
                )

            def hop2_group(gi):
                """h1 for hop-1 nodes [gi*128, (gi+1)*128)."""
                g2t = g2p.tile([128, NEI0 * F], F32, tag="g2t")
                for s in range(NEI0):
                    col = gi * NEI0 + s
                    gather128(g2t[:, s * F : (s + 1) * F], idx2_sb[:, col : col + 1])
                # sum 25 sample slices + x1 self term on PE: psum += I.T @ slice
                acc_ps = ppa.tile([128, F], F32, tag="acc")
                for s in range(NEI0):
                    nc.tensor.matmul(
                        acc_ps[:],
                        lhsT=ident[:],
                        rhs=g2t[:, s * F : (s + 1) * F],
                        start=(s == 0),
                        stop=False,
                    )
                nc.tensor.matmul(
                    acc_ps[:],
                    lhsT=ident[:],
                    rhs=x1_all[:, gi * F : (gi + 1) * F],
                    start=False,
                    stop=True,
                )
                agg = aggp.tile([128, F], F32, tag="agg")
                nc.scalar.copy(agg[:], acc_ps[:])
                aggT_ps = ppt.tile([128, 128], F32, tag="tp")
                nc.tensor.transpose(out=aggT_ps[:], in_=agg[:], identity=ident[:])
                aggT = aggp.tile([128, 128], F32, tag="aggT")
                nc.scalar.copy(aggT[:], aggT_ps[:])
                h1_ps = ppm.tile([128, E1], F32, tag="mm")
                nc.tensor.matmul(
                    h1_ps[:], lhsT=aggT[:], rhs=w0_sb[:], start=True, stop=True
                )
                nc.scalar.activation(
                    h1_all[:, gi * E1 : (gi + 1) * E1],
                    h1_ps[:],
                    mybir.ActivationFunctionType.Tanh,
                )

            def seed_group(G):
                """h0, layer-1 heads, decoder + softmax for seeds [G*128,(G+1)*128)."""
                # ---- h0: mean over 10 hop-1 gc rows (+2.5 scale), + x0, @W0, tanh
                ps0 = ppa.tile([128, F], F32, tag="acc")
                for t in range(TPG):
                    gi = G * TPG + t
                    nc.tensor.matmul(
                        ps0[:],
                        lhsT=sel_sb[:, t * 128 : (t + 1) * 128],
                        rhs=g1_all[:, gi * F : (gi + 1) * F],
                        start=(t == 0),
                        stop=(t == TPG - 1),
                    )
                tmp0 = smp.tile([128, F], F32, tag="tmp0")
                nc.vector.tensor_scalar(
                    out=tmp0[:],
                    in0=ps0[:],
                    scalar1=2.5,
                    scalar2=None,
                    op0=mybir.AluOpType.mult,
                )
                agg0 = smp.tile([128, F], F32, tag="agg0")
                nc.vector.tensor_add(
                    out=agg0[:], in0=tmp0[:], in1=x0_all[:, G * F : (G + 1) * F]
                )
                agg0T_ps = ppt.tile([128, 128], F32, tag="tp")
                nc.tensor.transpose(out=agg0T_ps[:], in_=agg0[:], identity=ident[:])
                agg0T = smp.tile([128, 128], F32, tag="agg0T")
                nc.scalar.copy(agg0T[:], agg0T_ps[:])
                h0_ps = ppm.tile([128, E1], F32, tag="mm")
                nc.tensor.matmul(
                    h0_ps[:], lhsT=agg0T[:], rhs=w0_sb[:], start=True, stop=True
                )
                nc.scalar.activation(
                    h0_all[:, G * E1 : (G + 1) * E1],
                    h0_ps[:],
                    mybir.ActivationFunctionType.Tanh,
                )
                # ---- layer 1: a1 = h0 + sum_t (sel*w1col).T @ h1
                ps1 = ppa.tile([128, E1], F32, tag="acc")
                for t in range(TPG):
                    gi = G * TPG + t
                    wsel = wselp.tile([128, 128], F32, tag="wsel")
                    nc.scalar.mul(
                        wsel[:],
                        sel_sb[:, t * 128 : (t + 1) * 128],
                        w1col_sb[:, gi : gi + 1],
                    )
                    nc.tensor.matmul(
                        ps1[:],
                        lhsT=wsel[:],
                        rhs=h1_all[:, gi * E1 : (gi + 1) * E1],
                        start=(t == 0),
                        stop=(t == TPG - 1),
                    )
                a1 = smp.tile([128, E1], F32, tag="a1")
                nc.vector.tensor_add(
                    out=a1[:], in0=ps1[:], in1=h0_all[:, G * E1 : (G + 1) * E1]
                )
                a1T_ps = ppt.tile([128, 128], F32, tag="tp")
                nc.tensor.transpose(out=a1T_ps[:], in_=a1[:], identity=ident[:])
                a1T = smp.tile([128, 128], F32, tag="a1T")
                nc.scalar.copy(a1T[:], a1T_ps[:])
                zm_ps = ppm.tile([128, E2], F32, tag="mm")
                nc.tensor.matmul(zm_ps[:], lhsT=a1T[:], rhs=wm_sb[:], start=True, stop=True)
                zs_ps = ppm.tile([128, E2], F32, tag="mm")
                nc.tensor.matmul(zs_ps[:], lhsT=a1T[:], rhs=ws_sb[:], start=True, stop=True)
                ezs = smp.tile([128, E2], F32, tag="ezs")
                nc.scalar.activation(ezs[:], zs_ps[:], mybir.ActivationFunctionType.Exp)
                zt = smp.tile([128, E2], F32, tag="zt")
                nc.vector.tensor_mul(
                    out=zt[:], in0=ezs[:], in1=eps_sb[:, G * E2 : (G + 1) * E2]
                )
                z = smp.tile([128, E2], F32, tag="z")
                nc.vector.tensor_add(out=z[:], in0=zm_ps[:], in1=zt[:])
                # ---- decoder: relu(z@W1+b1)@W2 + b2, softmax
                zT_ps = ppt.tile([E2, 128], F32, tag="tp")
                nc.tensor.transpose(out=zT_ps[:], in_=z[:], identity=ident[:])
                zT = smp.tile([E2, 128], F32, tag="zTs")
                nc.scalar.copy(zT[:], zT_ps[:])
                lg_ps = ppa.tile([128, C], F32, tag="acc")
                for ch in range(2):
                    r_ps = ppm.tile([128, 128], F32, tag="mm")
                    nc.tensor.matmul(
                        r_ps[:],
                        lhsT=w1_sb[:, ch * 128 : (ch + 1) * 128],
                        rhs=zT[:],
                        start=True,
                        stop=True,
                    )
                    r_sb = smp.tile([128, 128], F32, tag="rsb")
                    nc.scalar.activation(
                        r_sb[:],
                        r_ps[:],
                        mybir.ActivationFunctionType.Relu,
                        bias=b1_sb[:, ch : ch + 1],
                    )
                    nc.tensor.matmul(
                        lg_ps[:],
                        lhsT=r_sb[:],
                        rhs=w2_sb[:, ch * C : (ch + 1) * C],
                        start=(ch == 0),
                        stop=(ch == 1),
                    )
                lg = smp.tile([128, C], F32, tag="lgsb")
                nc.vector.tensor_add(out=lg[:], in0=lg_ps[:], in1=b2_sb[:])
                nm = smp.tile([128, 1], F32, tag="nm")
                nc.vector.reduce_max(
                    out=nm[:], in_=lg[:], axis=mybir.AxisListType.X, negate=True
                )
                ex = smp.tile([128, C], F32, tag="ex")
                nc.scalar.activation(
                    ex[:], lg[:], mybir.ActivationFunctionType.Exp, bias=nm[:, 0:1]
                )
                ssum = smp.tile([128, 1], F32, tag="ssum")
                nc.vector.reduce_sum(out=ssum[:], in_=ex[:], axis=mybir.AxisListType.X)
                rinv = smp.tile([128, 1], F32, tag="rinv")
                nc.vector.reciprocal(rinv[:], ssum[:])
                o_sb = smp.tile([128, C], F32, tag="osb")
                nc.vector.tensor_scalar(
                    out=o_sb[:],
                    in0=ex[:],
                    scalar1=rinv[:, 0:1],
                    scalar2=None,
                    op0=mybir.AluOpType.mult,
                )
                nc.sync.dma_start(out=out_d[G * 128 : (G + 1) * 128, :], in_=o_sb[:])

            # interleave: 10 hop-2 groups, then their seed group
            for G in range(SG):
                for t in range(TPG):
                    hop2_group(G * TPG + t)
                seed_group(G)

    nc.compile()
    return nc


def _get_nc():
    global _NC
    if _NC is None:
        _NC = _build_nc()
    return _NC


def _host_prep(nodes, adj_info, degrees, features, W0, Wm, Ws, W1, b1, W2, b2):
    """Replicate reference RNG + build per-core input maps."""
    import jax

    nodes = np.asarray(nodes).astype(np.int64)
    adj_info = np.asarray(adj_info).astype(np.int64)
    degrees = np.asarray(degrees, dtype=np.float32)
    features = np.asarray(features, dtype=np.float32)
    W0 = np.ascontiguousarray(np.asarray(W0, dtype=np.float32))
    Wm = np.ascontiguousarray(np.asarray(Wm, dtype=np.float32))
    Ws = np.ascontiguousarray(np.asarray(Ws, dtype=np.float32))
    W1 = np.ascontiguousarray(np.asarray(W1, dtype=np.float32))
    b1 = np.asarray(b1, dtype=np.float32)
    W2 = np.asarray(W2, dtype=np.float32)
    b2 = np.asarray(b2, dtype=np.float32)

    with jax.default_device(jax.devices("cpu")[0]):
        key = jax.random.key(42)
        k1, k2, k3 = jax.random.split(key, 3)
        p1 = np.asarray(jax.random.permutation(k1, MAXDEG))[:NEI1].astype(np.int64)
        p2 = np.asarray(jax.random.permutation(k2, MAXDEG))[:NEI0].astype(np.int64)
        eps = np.asarray(
            jax.random.normal(k3, (B, E2), dtype=np.float32), dtype=np.float32
        )

    gc = (features / (NEI0 * degrees[:, None])).astype(np.float32)
    gc = np.ascontiguousarray(gc)

    # selection matrix: sel[i, t*128+s] = 2.5 if (t*128+i)//10 == s  -> NO:
    # scale 2.5 applied on-device; entries are 0/1
    sel = np.zeros((128, TPG * 128), dtype=np.float32)
    for t in range(TPG):
        i = np.arange(128)
        s = (t * 128 + i) // NEI1
        sel[i, t * 128 + s] = 1.0

    w2r = np.zeros((128, 2 * C), dtype=np.float32)
    w2r[:, :C] = W2[:128]
    w2r[:, C:] = W2[128:]
    b1r = np.stack([b1[:128], b1[128:]], axis=1).astype(np.float32)
    b1r = np.ascontiguousarray(b1r)
    b2r = np.broadcast_to(b2[None, :], (128, C)).astype(np.float32)
    b2r = np.ascontiguousarray(b2r)

    in_maps = []
    for c in range(NCORES):
        nodes_c = nodes[c * BC : (c + 1) * BC]
        nb1 = adj_info[nodes_c][:, p1].reshape(-1)  # [5120]
        nb2 = adj_info[nb1[:, None], p2[None, :]]  # [5120, 25]

        idx1 = nb1.reshape(NG, 128).T.astype(np.int32)  # [128, NG]
        idx0 = nodes_c.reshape(SG, 128).T.astype(np.int32)  # [128, SG]
        # idx2[p, gi*25+s] = nb2[gi*128+p, s]
        idx2 = (
            nb2.reshape(NG, 128, NEI0).transpose(1, 0, 2).reshape(128, NG * NEI0)
        ).astype(np.int32)

        deg_nb1 = degrees[nb1]  # [5120]
        deg1s = (NEI0 * deg_nb1).reshape(NG, 128).T.astype(np.float32)
        w1col = (0.1 / deg_nb1).reshape(NG, 128).T.astype(np.float32)
        deg0s = (NEI0 * degrees[nodes_c]).reshape(SG, 128).T.astype(np.float32)

        eps_c = (
            eps[c * BC : (c + 1) * BC].reshape(SG, 128, E2).transpose(1, 0, 2)
        ).reshape(128, SG * E2)

        in_maps.append(
            {
                "gc": gc,
                "idx2": np.ascontiguousarray(idx2),
                "idx1": np.ascontiguousarray(idx1),
                "idx0": np.ascontiguousarray(idx0),
                "deg1s": np.ascontiguousarray(deg1s),
                "deg0s": np.ascontiguousarray(deg0s),
                "w1col": np.ascontiguousarray(w1col),
                "sel": sel,
                "eps": np.ascontiguousarray(eps_c.astype(np.float32)),
                "w0": W0,
                "wm": Wm,
                "ws": Ws,
                "w1": W1,
                "w2": w2r,
                "b1": b1r,
                "b2": b2r,
            }
        )
    return in_maps


def _run(in_maps, trace=False):
    nc = _get_nc()
    res = bass_utils.run_bass_kernel_spmd(
        nc, in_maps, core_ids=list(range(NCORES)), trace=trace
    )
    out = np.concatenate([r["out"] for r in res.results], axis=0)
    return out.astype(np.float32), res


def kernel(**inputs) -> np.ndarray:
    in_maps = _host_prep(**inputs)
    out, _ = _run(in_maps, trace=False)
    return out


def kernel_traced(**inputs):
    """Returns (output, exec_time_ns) using NTFF profiling."""
    in_maps = _host_prep(**inputs)
    out, res = _run(in_maps, trace=True)
    return out, res.exec_time_ns
